# revision 6
# baseline (speedup 1.0000x reference)
"""Trainium2 Bass kernel for the DiscretizedDPLRSSMBlock problem.

Computes, for h, x of shape [4096, 4096] (batch, hidden):

    out = h + (h * a_diag + (h @ q_vec) @ p_vec.T) + x @ b_mat        (DELTA = 1.0)
        = h * (1 + a_diag) + (h @ q_vec) @ p_vec.T + x @ b_mat

Sharding: data-parallel over the batch axis across 8 NeuronCores (512 rows
per core); a_diag/p_vec/q_vec/b_mat replicated.

Per-core kernel works in a transposed layout (hidden on partitions):
    outT[n, m] = sum_k B[k, n] * xT[k, m]        (x @ B, B tiles are the
                                                  stationary matmul operand
                                                  in natural DRAM layout)
               + sum_r p[n, r] * hqT[r, m]       (rank-4 term, hqT = q^T hT)
               + (1 + a[n]) * hT[n, m]           (per-partition scalar on DVE)

All matmul operands are bf16 (fp32 PSUM accumulation); output is fp32.
"""

import numpy as np
import ml_dtypes

import concourse.mybir as mybir
import concourse.tile as tile
from concourse import bacc
from concourse.bass_utils import run_bass_kernel_spmd

HIDDEN = 4096
BATCH = 4096
RANK = 4
N_CORES = 8
MB = BATCH // N_CORES  # 512 batch rows per core
P = 128
KT = HIDDEN // P       # 32 contraction tiles
NT = HIDDEN // P       # 32 output row tiles (hidden)
NCHUNK = 4             # resident tensors split into 4 DMA chunks
CH = KT // NCHUNK      # 8 k-tiles per chunk
NGROUP = NT // 8       # 4 n-tiles per b-column streaming group (512 cols)

BF16 = mybir.dt.bfloat16
F32 = mybir.dt.float32


def build_bass():
    """Build the single-core Tile program (same program runs SPMD on all 8)."""
    nc = bacc.Bacc("TRN2", target_bir_lowering=False, debug=False)

    b = nc.dram_tensor("b", [HIDDEN, HIDDEN], BF16, kind="ExternalInput")
    xT = nc.dram_tensor("xT", [HIDDEN, MB], BF16, kind="ExternalInput")
    hT = nc.dram_tensor("hT", [HIDDEN, MB], BF16, kind="ExternalInput")
    q = nc.dram_tensor("q", [HIDDEN, RANK], BF16, kind="ExternalInput")
    pT = nc.dram_tensor("pT", [RANK, HIDDEN], BF16, kind="ExternalInput")
    a_r = nc.dram_tensor("a_r", [P, NT], F32, kind="ExternalInput")
    outT = nc.dram_tensor("outT", [HIDDEN, MB], F32, kind="ExternalOutput")

    b_r = b.rearrange("(t p) n -> p t n", p=P)     # [128, 32, 4096]
    xT_r = xT.rearrange("(t p) m -> p t m", p=P)   # [128, 32, 512]
    hT_r = hT.rearrange("(t p) m -> p t m", p=P)
    q_r = q.rearrange("(t p) r -> p t r", p=P)     # [128, 32, 4]

    with (
        tile.TileContext(nc) as tc,
        tc.tile_pool(name="const", bufs=1) as cpool,
        tc.tile_pool(name="bcols", bufs=3) as bpool,
        tc.tile_pool(name="psum", bufs=6, space="PSUM") as pspool,
        tc.tile_pool(name="outs", bufs=4) as opool,
    ):
        # ---- DMA issue order is chosen to match PE consumption order ----
        # tier 1: b-group0 chunk / x chunk interleaved (PE's first 27us)
        xc = []
        bcs0 = []
        n_groups = NT // NGROUP
        for c in range(NCHUNK):
            bc = bpool.tile([P, CH, NGROUP * P], BF16, tag=f"b{c}")
            nc.sync.dma_start(bc[:], b_r[:, c * CH : (c + 1) * CH, 0 : NGROUP * P])
            bcs0.append(bc)
            xt = cpool.tile([P, CH, MB], BF16, tag=f"x{c}")
            nc.sync.dma_start(xt[:], xT_r[:, c * CH : (c + 1) * CH, :])
            xc.append(xt)
        # tier 2: hT (for hq + epilogues) + small constants
        hc = []
        for c in range(NCHUNK):
            ht = cpool.tile([P, CH, MB], BF16, tag=f"h{c}")
            nc.sync.dma_start(ht[:], hT_r[:, c * CH : (c + 1) * CH, :])
            hc.append(ht)
        q_sb = cpool.tile([P, KT, RANK], BF16, tag="q")
        nc.sync.dma_start(q_sb[:], q_r[:])
        pT_sb = cpool.tile([RANK, HIDDEN], BF16, tag="pT")
        nc.sync.dma_start(pT_sb[:], pT[:, :])
        araw = cpool.tile([P, NT], F32, tag="araw")
        nc.sync.dma_start(araw[:], a_r[:, :])
        a1 = cpool.tile([P, NT], F32, tag="a1")
        nc.vector.tensor_scalar_add(a1[:], araw[:], 1.0)

        def main_episodes(bcs, pss):
            # k-outer: episode c consumes exactly (b chunk c, x chunk c),
            # matching DMA delivery order; 4 psum banks accumulate.
            for c in range(NCHUNK):
                for sub in range(NGROUP):
                    for tt in range(CH):
                        nc.tensor.matmul(
                            pss[sub][:],
                            bcs[c][:, tt, sub * P : (sub + 1) * P],
                            xc[c][:, tt],
                            start=(c == 0 and tt == 0),
                            stop=False,
                        )

        def rank4_and_epilogue(g, pss, hq_sb):
            for sub in range(NGROUP):
                tn = g * NGROUP + sub
                nc.tensor.matmul(
                    pss[sub][:],
                    pT_sb[:, tn * P : (tn + 1) * P],
                    hq_sb[:],
                    start=False,
                    stop=True,
                )
            for sub in range(NGROUP):
                tn = g * NGROUP + sub
                ot = opool.tile([P, MB], F32, tag="ot")
                nc.vector.scalar_tensor_tensor(
                    ot[:],
                    hc[tn // CH][:, tn % CH],
                    a1[:, tn : tn + 1],
                    pss[sub][:],
                    mybir.AluOpType.mult,
                    mybir.AluOpType.add,
                )
                nc.sync.dma_start(outT[tn * P : (tn + 1) * P, :], ot[:])

        # ---- group 0: mains first (hq isn't ready yet) ----
        pss0 = [pspool.tile([P, MB], F32, tag="ps", name=f"ps0_{i}") for i in range(NGROUP)]
        main_episodes(bcs0, pss0)

        # hqT = q^T @ hT : [4, 512] (hT has landed by now)
        hq_ps = pspool.tile([RANK, MB], F32, tag="hq", bufs=1)
        for t in range(KT):
            nc.tensor.matmul(
                hq_ps[:],
                q_sb[:, t],
                hc[t // CH][:, t % CH],
                start=(t == 0),
                stop=(t == KT - 1),
            )
        hq_sb = cpool.tile([RANK, MB], BF16, tag="hq_sb")
        nc.vector.tensor_copy(hq_sb[:], hq_ps[:])

        rank4_and_epilogue(0, pss0, hq_sb)

        # ---- groups 1..7: inline ----
        for g in range(1, n_groups):
            n0 = g * NGROUP * P
            bcs = []
            for c in range(NCHUNK):
                bc = bpool.tile([P, CH, NGROUP * P], BF16, tag=f"b{c}")
                nc.sync.dma_start(
                    bc[:], b_r[:, c * CH : (c + 1) * CH, n0 : n0 + NGROUP * P]
                )
                bcs.append(bc)
            pss = [pspool.tile([P, MB], F32, tag="ps", name=f"ps{g}_{i}") for i in range(NGROUP)]
            main_episodes(bcs, pss)
            rank4_and_epilogue(g, pss, hq_sb)

    nc.compile()
    return nc


_NC_CACHE = []


def _get_nc():
    if not _NC_CACHE:
        _NC_CACHE.append(build_bass())
    return _NC_CACHE[0]


LAST_RESULTS = []  # stash of the last BassKernelResults, for test harnesses


def make_in_maps(h, x, a_diag, p_vec, q_vec, b_mat):
    """Shard + lay out the full inputs into per-core in_maps."""
    h = np.asarray(h, dtype=np.float32)
    x = np.asarray(x, dtype=np.float32)
    a_diag = np.asarray(a_diag, dtype=np.float32)
    p_vec = np.asarray(p_vec, dtype=np.float32)
    q_vec = np.asarray(q_vec, dtype=np.float32)
    b_mat = np.asarray(b_mat, dtype=np.float32)

    bf = ml_dtypes.bfloat16
    b_bf = np.ascontiguousarray(b_mat.astype(bf))
    q_bf = np.ascontiguousarray(q_vec.astype(bf))
    pT_bf = np.ascontiguousarray(p_vec.T.astype(bf))
    # a_r[p, t] = a_diag[t*128 + p]
    a_r = np.ascontiguousarray(a_diag.reshape(NT, P).T)

    in_maps = []
    for c in range(N_CORES):
        sl = slice(c * MB, (c + 1) * MB)
        in_maps.append(
            {
                "b": b_bf,
                "xT": np.ascontiguousarray(x[sl].T.astype(bf)),
                "hT": np.ascontiguousarray(h[sl].T.astype(bf)),
                "q": q_bf,
                "pT": pT_bf,
                "a_r": a_r,
            }
        )
    return in_maps


def _axon_device_reset():
    """Best-effort heal of a wedged axon-tunneled device (NRT_EXEC_UNIT_
    UNRECOVERABLE). No-op when the axon .so isn't present."""
    try:
        import ctypes

        lib = ctypes.CDLL("/opt/axon/libaxon_pjrt.so")
        lib.axon_reset.restype = ctypes.c_int64
        lib.axon_reset()
    except Exception:
        pass


def kernel(h, x, a_diag, p_vec, q_vec, b_mat, trace=False):
    nc = _get_nc()
    in_maps = make_in_maps(h, x, a_diag, p_vec, q_vec, b_mat)
    try:
        res = run_bass_kernel_spmd(
            nc, in_maps, core_ids=list(range(N_CORES)), trace=trace
        )
    except Exception as e:
        if "UNRECOVERABLE" not in str(e) and "UNAVAILABLE" not in str(e):
            raise
        _axon_device_reset()
        res = run_bass_kernel_spmd(
            nc, in_maps, core_ids=list(range(N_CORES)), trace=trace
        )
    LAST_RESULTS.clear()
    LAST_RESULTS.append(res)

    out = np.empty((BATCH, HIDDEN), dtype=np.float32)
    for c in range(N_CORES):
        out[c * MB : (c + 1) * MB, :] = res.results[c]["outT"].T
    return out


# revision 10
# speedup vs baseline: 1.0519x; 1.0519x over previous
"""Trainium2 Bass kernel for the DiscretizedDPLRSSMBlock problem.

Computes, for h, x of shape [4096, 4096] (batch, hidden):

    out = h + (h * a_diag + (h @ q_vec) @ p_vec.T) + x @ b_mat        (DELTA = 1.0)
        = h * (1 + a_diag) + (h @ q_vec) @ p_vec.T + x @ b_mat

Sharding: data-parallel over the batch axis across 8 NeuronCores (512 rows
per core); a_diag/p_vec/q_vec/b_mat replicated.

Per-core kernel works in a transposed layout (hidden on partitions):
    outT[n, m] = sum_k B[k, n] * xT[k, m]        (x @ B, B tiles are the
                                                  stationary matmul operand
                                                  in natural DRAM layout)
               + sum_r p[n, r] * hqT[r, m]       (rank-4 term, hqT = q^T hT)
               + (1 + a[n]) * hT[n, m]           (per-partition scalar on DVE)

All matmul operands are bf16 (fp32 PSUM accumulation); output is fp32.
"""

import numpy as np
import ml_dtypes

import concourse.mybir as mybir
import concourse.tile as tile
from concourse import bacc
from concourse.bass_utils import run_bass_kernel_spmd

HIDDEN = 4096
BATCH = 4096
RANK = 4
N_CORES = 8
MB = BATCH // N_CORES  # 512 batch rows per core
P = 128
KT = HIDDEN // P       # 32 contraction tiles
NT = HIDDEN // P       # 32 output row tiles (hidden)
NCHUNK = 4             # resident tensors split into 4 DMA chunks
CH = KT // NCHUNK      # 8 k-tiles per chunk
NGROUP = NT // 8       # 4 n-tiles per b-column streaming group (512 cols)

BF16 = mybir.dt.bfloat16
F32 = mybir.dt.float32


def build_bass():
    """Build the single-core Tile program (same program runs SPMD on all 8)."""
    nc = bacc.Bacc("TRN2", target_bir_lowering=False, debug=False)

    b = nc.dram_tensor("b", [HIDDEN, HIDDEN], BF16, kind="ExternalInput")
    xT = nc.dram_tensor("xT", [HIDDEN, MB], BF16, kind="ExternalInput")
    hT = nc.dram_tensor("hT", [HIDDEN, MB], BF16, kind="ExternalInput")
    q = nc.dram_tensor("q", [HIDDEN, RANK], BF16, kind="ExternalInput")
    pT = nc.dram_tensor("pT", [RANK, HIDDEN], BF16, kind="ExternalInput")
    a_r = nc.dram_tensor("a_r", [P, NT], F32, kind="ExternalInput")
    outT = nc.dram_tensor("outT", [HIDDEN, MB], F32, kind="ExternalOutput")

    b_r = b.rearrange("(t p) n -> p t n", p=P)     # [128, 32, 4096]
    xT_r = xT.rearrange("(t p) m -> p t m", p=P)   # [128, 32, 512]
    hT_r = hT.rearrange("(t p) m -> p t m", p=P)
    q_r = q.rearrange("(t p) r -> p t r", p=P)     # [128, 32, 4]

    # Episode chunking over the 32 k-tiles. 1MB-granularity transfers keep
    # the single HWDGE ring at full rate; finer chunks measured slower.
    CHUNKS = [(0, 8), (8, 8), (16, 8), (24, 8)]  # (t0, len)
    NEP = len(CHUNKS)

    with (
        tile.TileContext(nc) as tc,
        tc.tile_pool(name="const", bufs=1) as cpool,
        tc.tile_pool(name="bcols", bufs=3) as bpool,
        tc.tile_pool(name="psum", bufs=6, space="PSUM") as pspool,
        tc.tile_pool(name="outs", bufs=4) as opool,
    ):
        n_groups = NT // NGROUP

        def dma_b_group(g):
            n0 = g * NGROUP * P
            bcs = []
            for c, (t0, ln) in enumerate(CHUNKS):
                bc = bpool.tile(
                    [P, ln, NGROUP * P], BF16, tag=f"b{c}", name=f"b{g}_{c}"
                )
                nc.sync.dma_start(bc[:], b_r[:, t0 : t0 + ln, n0 : n0 + NGROUP * P])
                bcs.append(bc)
            return bcs

        # ---- DMA issue order chosen to match PE consumption order ----
        # Interleave b-group0 / x chunks (PE's first ~28us), with hT woven
        # into the tail so hq can start right when group 0's mains finish.
        # All input DMAs ride the Sync HWDGE ring; output DMAs ride the
        # Scalar ring so they can never head-of-line-block input streaming.
        xc, hc = [], []
        bcs0 = []
        for c, (t0, ln) in enumerate(CHUNKS):
            bc = bpool.tile([P, ln, NGROUP * P], BF16, tag=f"b{c}", name=f"b0_{c}")
            nc.sync.dma_start(bc[:], b_r[:, t0 : t0 + ln, 0 : NGROUP * P])
            bcs0.append(bc)
            xt = cpool.tile([P, ln, MB], BF16, tag=f"x{c}")
            nc.sync.dma_start(xt[:], xT_r[:, t0 : t0 + ln, :])
            xc.append(xt)
        for cc in range(4):
            ht = cpool.tile([P, CH, MB], BF16, tag=f"h{cc}")
            nc.sync.dma_start(ht[:], hT_r[:, cc * CH : (cc + 1) * CH, :])
            hc.append(ht)
        q_sb = cpool.tile([P, KT, RANK], BF16, tag="q")
        nc.sync.dma_start(q_sb[:], q_r[:])
        pT_sb = cpool.tile([RANK, HIDDEN], BF16, tag="pT")
        nc.sync.dma_start(pT_sb[:], pT[:, :])
        araw = cpool.tile([P, NT], F32, tag="araw")
        nc.sync.dma_start(araw[:], a_r[:, :])
        a1 = cpool.tile([P, NT], F32, tag="a1")
        nc.vector.tensor_scalar_add(a1[:], araw[:], 1.0)

        def sub_epilogue(tn, ps):
            ot = opool.tile([P, MB], F32, tag="ot", name=f"ot{tn}")
            nc.vector.scalar_tensor_tensor(
                ot[:],
                hc[tn // CH][:, tn % CH],
                a1[:, tn : tn + 1],
                ps[:],
                mybir.AluOpType.mult,
                mybir.AluOpType.add,
            )
            nc.scalar.dma_start(outT[tn * P : (tn + 1) * P, :], ot[:])

        def rank4(tn, ps):
            nc.tensor.matmul(
                ps[:],
                pT_sb[:, tn * P : (tn + 1) * P],
                hq_sb[:],
                start=False,
                stop=True,
            )

        def main_episodes(g, bcs, pss, tail_inline):
            # k-outer: episode c consumes exactly (b chunk c, x chunk c),
            # matching DMA delivery order; 4 psum banks accumulate.
            for c, (t0, ln) in enumerate(CHUNKS):
                last = c == NEP - 1
                for sub in range(NGROUP):
                    for tt in range(ln):
                        nc.tensor.matmul(
                            pss[sub][:],
                            bcs[c][:, tt, sub * P : (sub + 1) * P],
                            xc[c][:, tt],
                            start=(c == 0 and tt == 0),
                            stop=False,
                        )
                    if last and tail_inline:
                        tn = g * NGROUP + sub
                        rank4(tn, pss[sub])
                        sub_epilogue(tn, pss[sub])

        # ---- group 0: mains first (hq isn't ready yet) ----
        pss0 = [
            pspool.tile([P, MB], F32, tag="ps", name=f"ps0_{i}")
            for i in range(NGROUP)
        ]
        main_episodes(0, bcs0, pss0, tail_inline=False)

        # hqT = q^T @ hT : [4, 512] (hT has landed by now)
        hq_ps = pspool.tile([RANK, MB], F32, tag="hq", bufs=1)
        for t in range(KT):
            nc.tensor.matmul(
                hq_ps[:],
                q_sb[:, t],
                hc[t // CH][:, t % CH],
                start=(t == 0),
                stop=(t == KT - 1),
            )
        hq_sb = cpool.tile([RANK, MB], BF16, tag="hq_sb")
        nc.vector.tensor_copy(hq_sb[:], hq_ps[:])

        for sub in range(NGROUP):
            rank4(sub, pss0[sub])
        for sub in range(NGROUP):
            sub_epilogue(sub, pss0[sub])

        # ---- groups 1..7: inline rank4 + epilogue in the last episode ----
        for g in range(1, n_groups):
            bcs = dma_b_group(g)
            pss = [
                pspool.tile([P, MB], F32, tag="ps", name=f"ps{g}_{i}")
                for i in range(NGROUP)
            ]
            main_episodes(g, bcs, pss, tail_inline=True)

    nc.compile()
    return nc


_NC_CACHE = []


def _get_nc():
    if not _NC_CACHE:
        _NC_CACHE.append(build_bass())
    return _NC_CACHE[0]


LAST_RESULTS = []  # stash of the last BassKernelResults, for test harnesses


def make_in_maps(h, x, a_diag, p_vec, q_vec, b_mat):
    """Shard + lay out the full inputs into per-core in_maps."""
    h = np.asarray(h, dtype=np.float32)
    x = np.asarray(x, dtype=np.float32)
    a_diag = np.asarray(a_diag, dtype=np.float32)
    p_vec = np.asarray(p_vec, dtype=np.float32)
    q_vec = np.asarray(q_vec, dtype=np.float32)
    b_mat = np.asarray(b_mat, dtype=np.float32)

    bf = ml_dtypes.bfloat16
    b_bf = np.ascontiguousarray(b_mat.astype(bf))
    q_bf = np.ascontiguousarray(q_vec.astype(bf))
    pT_bf = np.ascontiguousarray(p_vec.T.astype(bf))
    # a_r[p, t] = a_diag[t*128 + p]
    a_r = np.ascontiguousarray(a_diag.reshape(NT, P).T)

    in_maps = []
    for c in range(N_CORES):
        sl = slice(c * MB, (c + 1) * MB)
        in_maps.append(
            {
                "b": b_bf,
                "xT": np.ascontiguousarray(x[sl].T.astype(bf)),
                "hT": np.ascontiguousarray(h[sl].T.astype(bf)),
                "q": q_bf,
                "pT": pT_bf,
                "a_r": a_r,
            }
        )
    return in_maps


def _axon_device_reset():
    """Best-effort heal of a wedged axon-tunneled device (NRT_EXEC_UNIT_
    UNRECOVERABLE). No-op when the axon .so isn't present."""
    try:
        import ctypes

        lib = ctypes.CDLL("/opt/axon/libaxon_pjrt.so")
        lib.axon_reset.restype = ctypes.c_int64
        lib.axon_reset()
    except Exception:
        pass


def kernel(h, x, a_diag, p_vec, q_vec, b_mat, trace=False):
    nc = _get_nc()
    in_maps = make_in_maps(h, x, a_diag, p_vec, q_vec, b_mat)
    try:
        res = run_bass_kernel_spmd(
            nc, in_maps, core_ids=list(range(N_CORES)), trace=trace
        )
    except Exception as e:
        if "UNRECOVERABLE" not in str(e) and "UNAVAILABLE" not in str(e):
            raise
        _axon_device_reset()
        res = run_bass_kernel_spmd(
            nc, in_maps, core_ids=list(range(N_CORES)), trace=trace
        )
    LAST_RESULTS.clear()
    LAST_RESULTS.append(res)

    out = np.empty((BATCH, HIDDEN), dtype=np.float32)
    for c in range(N_CORES):
        out[c * MB : (c + 1) * MB, :] = res.results[c]["outT"].T
    return out


# revision 12
# speedup vs baseline: 20.9558x; 19.9223x over previous
"""Trainium2 Bass kernel for the DiscretizedDPLRSSMBlock problem.

Computes, for h, x of shape [4096, 4096] (batch, hidden):

    out = h + (h * a_diag + (h @ q_vec) @ p_vec.T) + x @ b_mat        (DELTA = 1.0)
        = h * (1 + a_diag) + (h @ q_vec) @ p_vec.T + x @ b_mat

Sharding: data-parallel over the batch axis across 8 NeuronCores (512 rows
per core); a_diag/p_vec/q_vec/b_mat replicated.

Per-core kernel works in a transposed layout (hidden on partitions):
    outT[n, m] = sum_k B[k, n] * xT[k, m]        (x @ B, B tiles are the
                                                  stationary matmul operand
                                                  in natural DRAM layout)
               + sum_r p[n, r] * hqT[r, m]       (rank-4 term, hqT = q^T hT)
               + (1 + a[n]) * hT[n, m]           (per-partition scalar on DVE)

All matmul operands are bf16 (fp32 PSUM accumulation); output is fp32.
"""

import numpy as np
import ml_dtypes

import concourse.mybir as mybir
import concourse.tile as tile
from concourse import bacc
from concourse.bass_utils import run_bass_kernel_spmd

HIDDEN = 4096
BATCH = 4096
RANK = 4
N_CORES = 8
MB = BATCH // N_CORES  # 512 batch rows per core
P = 128
KT = HIDDEN // P       # 32 contraction tiles
NT = HIDDEN // P       # 32 output row tiles (hidden)
NCHUNK = 4             # resident tensors split into 4 DMA chunks
CH = KT // NCHUNK      # 8 k-tiles per chunk
NGROUP = NT // 8       # 4 n-tiles per b-column streaming group (512 cols)

BF16 = mybir.dt.bfloat16
F32 = mybir.dt.float32


def build_bass():
    """Build the single-core Tile program (same program runs SPMD on all 8)."""
    nc = bacc.Bacc("TRN2", target_bir_lowering=False, debug=False)

    b = nc.dram_tensor("b", [HIDDEN, HIDDEN], BF16, kind="ExternalInput")
    xT = nc.dram_tensor("xT", [HIDDEN, MB], BF16, kind="ExternalInput")
    hT = nc.dram_tensor("hT", [HIDDEN, MB], BF16, kind="ExternalInput")
    q = nc.dram_tensor("q", [HIDDEN, RANK], BF16, kind="ExternalInput")
    pT = nc.dram_tensor("pT", [RANK, HIDDEN], BF16, kind="ExternalInput")
    a_r = nc.dram_tensor("a_r", [P, NT], F32, kind="ExternalInput")
    outT = nc.dram_tensor("outT", [HIDDEN, MB], F32, kind="ExternalOutput")

    b_r = b.rearrange("(t p) n -> p t n", p=P)     # [128, 32, 4096]
    xT_r = xT.rearrange("(t p) m -> p t m", p=P)   # [128, 32, 512]
    hT_r = hT.rearrange("(t p) m -> p t m", p=P)
    q_r = q.rearrange("(t p) r -> p t r", p=P)     # [128, 32, 4]

    # Episode chunking over the 32 k-tiles. 1MB-granularity transfers keep
    # the single HWDGE ring at full rate; finer chunks measured slower.
    CHUNKS = [(0, 8), (8, 8), (16, 8), (24, 8)]  # (t0, len)
    NEP = len(CHUNKS)

    with (
        tile.TileContext(nc) as tc,
        tc.tile_pool(name="const", bufs=1) as cpool,
        tc.tile_pool(name="bcols", bufs=3) as bpool,
        tc.tile_pool(name="psum", bufs=6, space="PSUM") as pspool,
        tc.tile_pool(name="outs", bufs=4) as opool,
    ):
        n_groups = NT // NGROUP

        def dma_b_group(g):
            n0 = g * NGROUP * P
            bcs = []
            for c, (t0, ln) in enumerate(CHUNKS):
                bc = bpool.tile(
                    [P, ln, NGROUP * P], BF16, tag=f"b{c}", name=f"b{g}_{c}"
                )
                nc.sync.dma_start(bc[:], b_r[:, t0 : t0 + ln, n0 : n0 + NGROUP * P])
                bcs.append(bc)
            return bcs

        # ---- DMA issue order chosen to match PE consumption order ----
        # Interleave b-group0 / x chunks (PE's first ~28us), with hT woven
        # into the tail so hq can start right when group 0's mains finish.
        # All input DMAs ride the Sync HWDGE ring; output DMAs ride the
        # Scalar ring so they can never head-of-line-block input streaming.
        # Issue order: pure b/x for the first two episodes, then weave hT
        # chunks (for the hq prologue) into the tail — each lands just
        # before its interleaved hq chunk-matmuls need it.
        xc, hc = [], []
        bcs0 = []

        def dma_x(c):
            t0, ln = CHUNKS[c]
            xt = cpool.tile([P, ln, MB], BF16, tag=f"x{c}")
            nc.sync.dma_start(xt[:], xT_r[:, t0 : t0 + ln, :])
            xc.append(xt)

        def dma_b0(c):
            t0, ln = CHUNKS[c]
            bc = bpool.tile([P, ln, NGROUP * P], BF16, tag=f"b{c}", name=f"b0_{c}")
            nc.sync.dma_start(bc[:], b_r[:, t0 : t0 + ln, 0 : NGROUP * P])
            bcs0.append(bc)

        def dma_h(cc):
            ht = cpool.tile([P, CH, MB], BF16, tag=f"h{cc}", name=f"h{cc}")
            nc.sync.dma_start(ht[:], hT_r[:, cc * CH : (cc + 1) * CH, :])
            hc.append(ht)

        dma_b0(0); dma_x(0); dma_b0(1); dma_x(1)
        dma_h(0)
        q_sb = cpool.tile([P, KT, RANK], BF16, tag="q")
        nc.sync.dma_start(q_sb[:], q_r[:])
        dma_b0(2); dma_x(2)
        dma_h(1)
        dma_b0(3); dma_x(3)
        dma_h(2); dma_h(3)
        pT_sb = cpool.tile([RANK, HIDDEN], BF16, tag="pT")
        nc.sync.dma_start(pT_sb[:], pT[:, :])
        araw = cpool.tile([P, NT], F32, tag="araw")
        nc.sync.dma_start(araw[:], a_r[:, :])
        a1 = cpool.tile([P, NT], F32, tag="a1")
        nc.vector.tensor_scalar_add(a1[:], araw[:], 1.0)

        def sub_epilogue(tn, ps):
            ot = opool.tile([P, MB], F32, tag="ot", name=f"ot{tn}")
            nc.vector.scalar_tensor_tensor(
                ot[:],
                hc[tn // CH][:, tn % CH],
                a1[:, tn : tn + 1],
                ps[:],
                mybir.AluOpType.mult,
                mybir.AluOpType.add,
            )
            nc.scalar.dma_start(outT[tn * P : (tn + 1) * P, :], ot[:])

        def rank4(tn, ps):
            nc.tensor.matmul(
                ps[:],
                pT_sb[:, tn * P : (tn + 1) * P],
                hq_sb[:],
                start=False,
                stop=True,
            )

        def main_episodes(g, bcs, pss, tail_inline):
            # k-outer: episode c consumes exactly (b chunk c, x chunk c),
            # matching DMA delivery order; 4 psum banks accumulate.
            for c, (t0, ln) in enumerate(CHUNKS):
                last = c == NEP - 1
                for sub in range(NGROUP):
                    for tt in range(ln):
                        nc.tensor.matmul(
                            pss[sub][:],
                            bcs[c][:, tt, sub * P : (sub + 1) * P],
                            xc[c][:, tt],
                            start=(c == 0 and tt == 0),
                            stop=False,
                        )
                    if last and tail_inline:
                        tn = g * NGROUP + sub
                        rank4(tn, pss[sub])
                        sub_epilogue(tn, pss[sub])

        # ---- group 0: mains with the hq prologue (hqT = q^T @ hT, [4,512])
        # chunk-interleaved between episodes as each hT chunk lands ----
        pss0 = [
            pspool.tile([P, MB], F32, tag="ps", name=f"ps0_{i}")
            for i in range(NGROUP)
        ]
        hq_ps = pspool.tile([RANK, MB], F32, tag="hq", bufs=1)

        def g0_episode(c):
            t0, ln = CHUNKS[c]
            for sub in range(NGROUP):
                for tt in range(ln):
                    nc.tensor.matmul(
                        pss0[sub][:],
                        bcs0[c][:, tt, sub * P : (sub + 1) * P],
                        xc[c][:, tt],
                        start=(c == 0 and tt == 0),
                        stop=False,
                    )

        def hq_chunk(cc):
            for tt in range(CH):
                nc.tensor.matmul(
                    hq_ps[:],
                    q_sb[:, cc * CH + tt],
                    hc[cc][:, tt],
                    start=(cc == 0 and tt == 0),
                    stop=(cc == 3 and tt == CH - 1),
                )

        g0_episode(0)
        g0_episode(1)
        hq_chunk(0)
        g0_episode(2)
        hq_chunk(1)
        g0_episode(3)
        hq_chunk(2)
        hq_chunk(3)

        hq_sb = cpool.tile([RANK, MB], BF16, tag="hq_sb")
        nc.vector.tensor_copy(hq_sb[:], hq_ps[:])

        for sub in range(NGROUP):
            rank4(sub, pss0[sub])
        for sub in range(NGROUP):
            sub_epilogue(sub, pss0[sub])

        # ---- groups 1..7: inline rank4 + epilogue in the last episode ----
        for g in range(1, n_groups):
            bcs = dma_b_group(g)
            pss = [
                pspool.tile([P, MB], F32, tag="ps", name=f"ps{g}_{i}")
                for i in range(NGROUP)
            ]
            main_episodes(g, bcs, pss, tail_inline=True)

    nc.compile()
    return nc


_NC_CACHE = []


def _get_nc():
    if not _NC_CACHE:
        _NC_CACHE.append(build_bass())
    return _NC_CACHE[0]


LAST_RESULTS = []  # stash of the last BassKernelResults, for test harnesses


def make_in_maps(h, x, a_diag, p_vec, q_vec, b_mat):
    """Shard + lay out the full inputs into per-core in_maps."""
    h = np.asarray(h, dtype=np.float32)
    x = np.asarray(x, dtype=np.float32)
    a_diag = np.asarray(a_diag, dtype=np.float32)
    p_vec = np.asarray(p_vec, dtype=np.float32)
    q_vec = np.asarray(q_vec, dtype=np.float32)
    b_mat = np.asarray(b_mat, dtype=np.float32)

    bf = ml_dtypes.bfloat16
    b_bf = np.ascontiguousarray(b_mat.astype(bf))
    q_bf = np.ascontiguousarray(q_vec.astype(bf))
    pT_bf = np.ascontiguousarray(p_vec.T.astype(bf))
    # a_r[p, t] = a_diag[t*128 + p]
    a_r = np.ascontiguousarray(a_diag.reshape(NT, P).T)

    in_maps = []
    for c in range(N_CORES):
        sl = slice(c * MB, (c + 1) * MB)
        in_maps.append(
            {
                "b": b_bf,
                "xT": np.ascontiguousarray(x[sl].T.astype(bf)),
                "hT": np.ascontiguousarray(h[sl].T.astype(bf)),
                "q": q_bf,
                "pT": pT_bf,
                "a_r": a_r,
            }
        )
    return in_maps


def _axon_device_reset():
    """Best-effort heal of a wedged axon-tunneled device (NRT_EXEC_UNIT_
    UNRECOVERABLE). No-op when the axon .so isn't present."""
    try:
        import ctypes

        lib = ctypes.CDLL("/opt/axon/libaxon_pjrt.so")
        lib.axon_reset.restype = ctypes.c_int64
        lib.axon_reset()
    except Exception:
        pass


def kernel(h, x, a_diag, p_vec, q_vec, b_mat, trace=False):
    nc = _get_nc()
    in_maps = make_in_maps(h, x, a_diag, p_vec, q_vec, b_mat)
    try:
        res = run_bass_kernel_spmd(
            nc, in_maps, core_ids=list(range(N_CORES)), trace=trace
        )
    except Exception as e:
        if "UNRECOVERABLE" not in str(e) and "UNAVAILABLE" not in str(e):
            raise
        _axon_device_reset()
        res = run_bass_kernel_spmd(
            nc, in_maps, core_ids=list(range(N_CORES)), trace=trace
        )
    LAST_RESULTS.clear()
    LAST_RESULTS.append(res)

    out = np.empty((BATCH, HIDDEN), dtype=np.float32)
    for c in range(N_CORES):
        out[c * MB : (c + 1) * MB, :] = res.results[c]["outT"].T
    return out


# revision 14
# speedup vs baseline: 21.0399x; 1.0040x over previous
"""Trainium2 Bass kernel for the DiscretizedDPLRSSMBlock problem.

Computes, for h, x of shape [4096, 4096] (batch, hidden):

    out = h + (h * a_diag + (h @ q_vec) @ p_vec.T) + x @ b_mat        (DELTA = 1.0)
        = h * (1 + a_diag) + (h @ q_vec) @ p_vec.T + x @ b_mat

Sharding: data-parallel over the batch axis across 8 NeuronCores (512 rows
per core); a_diag/p_vec/q_vec/b_mat replicated.

Per-core kernel works in a transposed layout (hidden on partitions):
    outT[n, m] = sum_k B[k, n] * xT[k, m]        (x @ B, B tiles are the
                                                  stationary matmul operand
                                                  in natural DRAM layout)
               + sum_r p[n, r] * hqT[r, m]       (rank-4 term, hqT = q^T hT)
               + (1 + a[n]) * hT[n, m]           (per-partition scalar on DVE)

All matmul operands are bf16 (fp32 PSUM accumulation); output is fp32.
"""

import numpy as np
import ml_dtypes

import concourse.mybir as mybir
import concourse.tile as tile
from concourse import bacc
from concourse.bass_utils import run_bass_kernel_spmd

HIDDEN = 4096
BATCH = 4096
RANK = 4
N_CORES = 8
MB = BATCH // N_CORES  # 512 batch rows per core
P = 128
KT = HIDDEN // P       # 32 contraction tiles
NT = HIDDEN // P       # 32 output row tiles (hidden)
NCHUNK = 4             # resident tensors split into 4 DMA chunks
CH = KT // NCHUNK      # 8 k-tiles per chunk
NGROUP = NT // 8       # 4 n-tiles per b-column streaming group (512 cols)

BF16 = mybir.dt.bfloat16
F32 = mybir.dt.float32


def build_bass():
    """Build the single-core Tile program (same program runs SPMD on all 8)."""
    nc = bacc.Bacc("TRN2", target_bir_lowering=False, debug=False)

    b = nc.dram_tensor("b", [HIDDEN, HIDDEN], BF16, kind="ExternalInput")
    xT = nc.dram_tensor("xT", [HIDDEN, MB], BF16, kind="ExternalInput")
    hT = nc.dram_tensor("hT", [HIDDEN, MB], BF16, kind="ExternalInput")
    q = nc.dram_tensor("q", [HIDDEN, RANK], BF16, kind="ExternalInput")
    pT = nc.dram_tensor("pT", [RANK, HIDDEN], BF16, kind="ExternalInput")
    a_r = nc.dram_tensor("a_r", [P, NT], F32, kind="ExternalInput")
    outT = nc.dram_tensor("outT", [HIDDEN, MB], F32, kind="ExternalOutput")

    b_r = b.rearrange("(t p) n -> p t n", p=P)     # [128, 32, 4096]
    xT_r = xT.rearrange("(t p) m -> p t m", p=P)   # [128, 32, 512]
    hT_r = hT.rearrange("(t p) m -> p t m", p=P)
    q_r = q.rearrange("(t p) r -> p t r", p=P)     # [128, 32, 4]

    # Episode chunking over the 32 k-tiles. 1MB-granularity transfers keep
    # the single HWDGE ring at full rate; finer chunks measured slower.
    CHUNKS = [(0, 8), (8, 8), (16, 8), (24, 8)]  # (t0, len)
    NEP = len(CHUNKS)

    with (
        tile.TileContext(nc) as tc,
        tc.tile_pool(name="const", bufs=1) as cpool,
        tc.tile_pool(name="bcols", bufs=3) as bpool,
        tc.tile_pool(name="psum", bufs=6, space="PSUM") as pspool,
        tc.tile_pool(name="outs", bufs=4) as opool,
    ):
        n_groups = NT // NGROUP

        def dma_b_group(g):
            n0 = g * NGROUP * P
            bcs = []
            for c, (t0, ln) in enumerate(CHUNKS):
                bc = bpool.tile(
                    [P, ln, NGROUP * P], BF16, tag=f"b{c}", name=f"b{g}_{c}"
                )
                nc.sync.dma_start(bc[:], b_r[:, t0 : t0 + ln, n0 : n0 + NGROUP * P])
                bcs.append(bc)
            return bcs

        # ---- DMA issue order chosen to match PE consumption order ----
        # Interleave b-group0 / x chunks (PE's first ~28us), with hT woven
        # into the tail so hq can start right when group 0's mains finish.
        # All input DMAs ride the Sync HWDGE ring; output DMAs ride the
        # Scalar ring so they can never head-of-line-block input streaming.
        # Issue order: pure b/x for the first two episodes, then weave hT
        # chunks (for the hq prologue) into the tail — each lands just
        # before its interleaved hq chunk-matmuls need it.
        xc, hc = [], []
        bcs0 = []

        def dma_x(c):
            t0, ln = CHUNKS[c]
            xt = cpool.tile([P, ln, MB], BF16, tag=f"x{c}")
            nc.sync.dma_start(xt[:], xT_r[:, t0 : t0 + ln, :])
            xc.append(xt)

        def dma_b0(c):
            t0, ln = CHUNKS[c]
            bc = bpool.tile([P, ln, NGROUP * P], BF16, tag=f"b{c}", name=f"b0_{c}")
            nc.sync.dma_start(bc[:], b_r[:, t0 : t0 + ln, 0 : NGROUP * P])
            bcs0.append(bc)

        def dma_h(cc):
            ht = cpool.tile([P, CH, MB], BF16, tag=f"h{cc}", name=f"h{cc}")
            nc.sync.dma_start(ht[:], hT_r[:, cc * CH : (cc + 1) * CH, :])
            hc.append(ht)

        dma_b0(0); dma_x(0); dma_b0(1); dma_x(1)
        dma_h(0)
        q_sb = cpool.tile([P, KT, RANK], BF16, tag="q")
        nc.sync.dma_start(q_sb[:], q_r[:])
        dma_b0(2); dma_x(2)
        dma_h(1)
        dma_b0(3); dma_x(3)
        dma_h(2); dma_h(3)
        pT_sb = cpool.tile([RANK, HIDDEN], BF16, tag="pT")
        nc.sync.dma_start(pT_sb[:], pT[:, :])
        araw = cpool.tile([P, NT], F32, tag="araw")
        nc.sync.dma_start(araw[:], a_r[:, :])
        a1 = cpool.tile([P, NT], F32, tag="a1")
        nc.vector.tensor_scalar_add(a1[:], araw[:], 1.0)

        def sub_epilogue(tn, ps):
            ot = opool.tile([P, MB], F32, tag="ot", name=f"ot{tn}")
            nc.vector.scalar_tensor_tensor(
                ot[:],
                hc[tn // CH][:, tn % CH],
                a1[:, tn : tn + 1],
                ps[:],
                mybir.AluOpType.mult,
                mybir.AluOpType.add,
            )
            nc.scalar.dma_start(outT[tn * P : (tn + 1) * P, :], ot[:])

        def rank4(tn, ps):
            nc.tensor.matmul(
                ps[:],
                pT_sb[:, tn * P : (tn + 1) * P],
                hq_sb[:],
                start=False,
                stop=True,
            )

        def main_episodes(g, bcs, pss, tail_inline):
            # k-outer: episode c consumes exactly (b chunk c, x chunk c),
            # matching DMA delivery order; 4 psum banks accumulate.
            for c, (t0, ln) in enumerate(CHUNKS):
                last = c == NEP - 1
                for sub in range(NGROUP):
                    for tt in range(ln):
                        nc.tensor.matmul(
                            pss[sub][:],
                            bcs[c][:, tt, sub * P : (sub + 1) * P],
                            xc[c][:, tt],
                            start=(c == 0 and tt == 0),
                            stop=False,
                        )
                    if last and tail_inline:
                        tn = g * NGROUP + sub
                        rank4(tn, pss[sub])
                        sub_epilogue(tn, pss[sub])

        # ---- group 0: mains with the hq prologue (hqT = q^T @ hT, [4,512])
        # chunk-interleaved between episodes as each hT chunk lands ----
        pss0 = [
            pspool.tile([P, MB], F32, tag="ps", name=f"ps0_{i}")
            for i in range(NGROUP)
        ]
        hq_ps = pspool.tile([RANK, MB], F32, tag="hq", bufs=1)

        def g0_episode(c):
            t0, ln = CHUNKS[c]
            for sub in range(NGROUP):
                for tt in range(ln):
                    nc.tensor.matmul(
                        pss0[sub][:],
                        bcs0[c][:, tt, sub * P : (sub + 1) * P],
                        xc[c][:, tt],
                        start=(c == 0 and tt == 0),
                        stop=False,
                    )

        def hq_chunk(cc):
            for tt in range(CH):
                nc.tensor.matmul(
                    hq_ps[:],
                    q_sb[:, cc * CH + tt],
                    hc[cc][:, tt],
                    start=(cc == 0 and tt == 0),
                    stop=(cc == 3 and tt == CH - 1),
                )

        g0_episode(0)
        g0_episode(1)
        hq_chunk(0)
        g0_episode(2)
        hq_chunk(1)
        g0_episode(3)
        hq_chunk(2)
        hq_chunk(3)

        hq_sb = cpool.tile([RANK, MB], BF16, tag="hq_sb")
        nc.vector.tensor_copy(hq_sb[:], hq_ps[:])

        for sub in range(NGROUP):
            rank4(sub, pss0[sub])
        for sub in range(NGROUP):
            sub_epilogue(sub, pss0[sub])

        # ---- groups 1..7: inline rank4 + epilogue in the last episode ----
        for g in range(1, n_groups):
            bcs = dma_b_group(g)
            pss = [
                pspool.tile([P, MB], F32, tag="ps", name=f"ps{g}_{i}")
                for i in range(NGROUP)
            ]
            main_episodes(g, bcs, pss, tail_inline=True)

    nc.compile()
    return nc


_NC_CACHE = []


def _get_nc():
    if not _NC_CACHE:
        _NC_CACHE.append(build_bass())
    return _NC_CACHE[0]


LAST_RESULTS = []  # stash of the last BassKernelResults, for test harnesses


def make_in_maps(h, x, a_diag, p_vec, q_vec, b_mat):
    """Shard + lay out the full inputs into per-core in_maps."""
    h = np.asarray(h, dtype=np.float32)
    x = np.asarray(x, dtype=np.float32)
    a_diag = np.asarray(a_diag, dtype=np.float32)
    p_vec = np.asarray(p_vec, dtype=np.float32)
    q_vec = np.asarray(q_vec, dtype=np.float32)
    b_mat = np.asarray(b_mat, dtype=np.float32)

    bf = ml_dtypes.bfloat16
    b_bf = np.ascontiguousarray(b_mat.astype(bf))
    q_bf = np.ascontiguousarray(q_vec.astype(bf))
    pT_bf = np.ascontiguousarray(p_vec.T.astype(bf))
    # a_r[p, t] = a_diag[t*128 + p]
    a_r = np.ascontiguousarray(a_diag.reshape(NT, P).T)

    in_maps = []
    for c in range(N_CORES):
        sl = slice(c * MB, (c + 1) * MB)
        in_maps.append(
            {
                "b": b_bf,
                "xT": np.ascontiguousarray(x[sl].T.astype(bf)),
                "hT": np.ascontiguousarray(h[sl].T.astype(bf)),
                "q": q_bf,
                "pT": pT_bf,
                "a_r": a_r,
            }
        )
    return in_maps


def _axon_device_reset():
    """Best-effort heal of a wedged axon-tunneled device (NRT_EXEC_UNIT_
    UNRECOVERABLE). No-op when the axon .so isn't present."""
    try:
        import ctypes

        lib = ctypes.CDLL("/opt/axon/libaxon_pjrt.so")
        lib.axon_reset.restype = ctypes.c_int64
        lib.axon_reset()
    except Exception:
        pass


def kernel(h, x, a_diag, p_vec, q_vec, b_mat, trace=False):
    nc = _get_nc()
    in_maps = make_in_maps(h, x, a_diag, p_vec, q_vec, b_mat)
    try:
        res = run_bass_kernel_spmd(
            nc, in_maps, core_ids=list(range(N_CORES)), trace=trace
        )
    except Exception as e:
        if "UNRECOVERABLE" not in str(e) and "UNAVAILABLE" not in str(e):
            raise
        _axon_device_reset()
        res = run_bass_kernel_spmd(
            nc, in_maps, core_ids=list(range(N_CORES)), trace=trace
        )
    LAST_RESULTS.clear()
    LAST_RESULTS.append(res)

    out = np.empty((BATCH, HIDDEN), dtype=np.float32)
    for c in range(N_CORES):
        out[c * MB : (c + 1) * MB, :] = res.results[c]["outT"].T
    return out
